# revision 9
# baseline (speedup 1.0000x reference)
"""GPT3 parallel attention block on 8 Trainium2 NeuronCores.

Tensor-parallel over heads: each of the 8 cores owns 2 of the 16 heads.
Per core: QKV projection for its 768 channels, causal attention for its
2 heads x 2 batches, and the dense projection restricted to its head
columns, producing a partial [H, B*S] output. Partials are summed on the
host (the all-reduce of the reference sharding).

Layouts (device, per core):
  xT      [H, B*S]   fp16  hidden states transposed; token t = b*S + s
  wqkvT   [H, 768]   fp16  qkv weight slice, channels [q0 k0 v0 q1 k1 v1]
  qkvb    [768]      fp32  qkv bias slice (same channel order)
  dwT     [256, H]   fp16  dense weight slice, rows = (head, d) in-channels
  maskm   [128, 896] fp16  sliding causal mask master
  outT    [H, B*S]   fp16  partial output (out-channel major)

All matmuls run in fp16 operands / fp32 PSUM accumulation. Softmax is
unnormalized exp (no max subtraction; scores are O(1)) with the
denominator computed by an ones-matmul that replicates the row sum
across all 128 partitions, so the normalization is a plain elementwise
multiply by the DVE reciprocal.
"""

import math

import numpy as np

S, B, H, NH, D = 2048, 2, 2048, 16, 128
NCORES = 8
CHUNK = 512
N_CHUNKS = S // CHUNK  # 4
K_TILES = H // 128  # 16
SCALE = 1.0 / math.sqrt(float(D))  # coeff / (sqrt(d) * coeff)

_CACHE: dict = {}


def _build_program():
    import concourse.tile as tile
    from concourse import bacc, mybir
    from concourse.masks import make_identity

    fp16 = mybir.dt.float16
    fp32 = mybir.dt.float32

    nc = bacc.Bacc(
        "TRN2",
        target_bir_lowering=False,
        debug=False,
        enable_asserts=True,
        num_devices=NCORES,
    )
    xT = nc.dram_tensor("xT", [H, B * S], fp16, kind="ExternalInput").ap()
    wq = nc.dram_tensor("wqkvT", [H, 768], fp16, kind="ExternalInput").ap()
    qb = nc.dram_tensor("qkvb", [768], fp32, kind="ExternalInput").ap()
    dw = nc.dram_tensor("dwT", [256, H], fp16, kind="ExternalInput").ap()
    mask = nc.dram_tensor("maskm", [128, 896], fp16, kind="ExternalInput").ap()
    outT = nc.dram_tensor("outT", [H, B * S], fp16, kind="ExternalOutput").ap()

    with tile.TileContext(nc) as tc:
        with (
            tc.tile_pool(name="singles", bufs=1) as singles,
            tc.tile_pool(name="xk", bufs=36) as x_pool,
            tc.tile_pool(name="qt", bufs=4) as qt_pool,
            tc.tile_pool(name="kt", bufs=4) as kt_pool,
            tc.tile_pool(name="vv", bufs=4) as v_pool,
            tc.tile_pool(name="vt", bufs=3) as vt_pool,
            tc.tile_pool(name="pt", bufs=20) as pt_pool,
            tc.tile_pool(name="rec", bufs=2) as rec_pool,
            tc.tile_pool(name="cx", bufs=4) as cx_pool,
            tc.tile_pool(name="ost", bufs=6) as ost_pool,
            tc.tile_pool(name="ps_qkv", bufs=2, space="PSUM") as ps_qkv,
            tc.tile_pool(name="ps_misc", bufs=2, space="PSUM") as ps_misc,
            tc.tile_pool(name="ps_sc", bufs=2, space="PSUM") as ps_sc,
            tc.tile_pool(name="ps_ctx", bufs=2, space="PSUM") as ps_ctx,
        ):
            # --- one-time loads / constants (weight k-tiles split so the
            # first QKV accumulation can start before the full load lands)
            w_all = singles.tile([128, K_TILES, 768], fp16, tag="w_all")
            dw_all = singles.tile([128, 2, H], fp16, tag="dw_all")
            mask_t = singles.tile([128, 896], fp16, tag="mask_t")
            qb_t = singles.tile([128, 6], fp32, tag="qb_t")
            ident = singles.tile([128, 128], fp32, tag="ident")
            ones_t = singles.tile([128, 128], fp16, tag="ones_t")

            wq_v = wq.rearrange("(k p) c -> p k c", p=128)

            Ident = mybir.ActivationFunctionType.Identity
            Exp = mybir.ActivationFunctionType.Exp

            kT = {}
            Vb = {}
            qt = {}
            state = {}
            pt_gen = [0]  # first pass through the pt pool must write full tiles

            def load_x(b, j):
                tok0 = b * S + j * CHUNK
                xk = []
                for k in range(K_TILES):
                    xt = x_pool.tile([128, CHUNK], fp16, tag="xk", name="xk")
                    nc.sync.dma_start(
                        out=xt,
                        in_=xT[k * 128 : (k + 1) * 128, tok0 : tok0 + CHUNK],
                    )
                    xk.append(xt)
                return xk

            def stage1(b, j, xk):
                # q, k, v for chunk j of batch b, both heads, then V transpose
                if j == 0:
                    kT[b] = [
                        kt_pool.tile([128, S], fp16, tag="kt", name="kt")
                        for _ in range(2)
                    ]
                    Vb[b] = [
                        v_pool.tile([128, S], fp16, tag="vv", name="vv")
                        for _ in range(2)
                    ]
                vt = []
                qt[(b, j)] = []
                for h in range(2):
                    qtile = qt_pool.tile([128, CHUNK], fp16, tag="qt", name="qt")
                    vtile = vt_pool.tile([128, CHUNK], fp32, tag="vt", name="vt")
                    qt[(b, j)].append(qtile)
                    vt.append(vtile)
                    for which in range(3):  # q, k, v
                        ci = 3 * h + which
                        ps = ps_qkv.tile([128, CHUNK], fp32, tag="ps_qkv", name="ps")
                        for k in range(K_TILES):
                            nc.tensor.matmul(
                                out=ps,
                                lhsT=w_all[:, k, ci * 128 : (ci + 1) * 128],
                                rhs=xk[k],
                                start=(k == 0),
                                stop=(k == K_TILES - 1),
                            )
                        if which == 0:
                            dest = qtile
                        elif which == 1:
                            dest = kT[b][h][:, j * CHUNK : (j + 1) * CHUNK]
                        else:
                            dest = vtile
                        nc.vector.tensor_scalar_add(
                            out=dest, in0=ps, scalar1=qb_t[:, ci : ci + 1]
                        )
                for h in range(2):
                    tp = ps_misc.tile([128, CHUNK], fp32, tag="ps_misc", name="tp")
                    for ti in range(4):
                        nc.tensor.transpose(
                            out=tp[:, ti * 128 : (ti + 1) * 128],
                            in_=vt[h][:, ti * 128 : (ti + 1) * 128],
                            identity=ident,
                        )
                    nc.vector.tensor_copy(
                        out=Vb[b][h][:, j * CHUNK : (j + 1) * CHUNK], in_=tp
                    )

            def attn_a(b, j):
                accs = []
                for h in range(2):
                    n_t = 4 * j + 4
                    ctx = ps_ctx.tile([128, CHUNK], fp32, tag="ps_ctx", name="ctx")
                    pts = []
                    psums = []
                    for i in range(n_t):
                        # diagonal tiles only need columns >= r; the masked
                        # rest of pt is zeroed by the mask multiply. The first
                        # generation of each pool slot must be written fully
                        # (stale SBUF can hold inf/NaN bit patterns).
                        rm = (i - 4 * j) * 128 if i >= 4 * j else 0
                        r = 0 if pt_gen[0] < 20 else rm
                        pt_gen[0] += 1
                        sc = ps_sc.tile([128, CHUNK], fp32, tag="ps_sc", name="sc")
                        nc.tensor.matmul(
                            out=sc[:, r:CHUNK],
                            lhsT=kT[b][h][:, i * 128 : (i + 1) * 128],
                            rhs=qt[(b, j)][h][:, r:CHUNK],
                            start=True,
                            stop=True,
                        )
                        pt = pt_pool.tile([128, CHUNK], fp16, tag="pt", name="pt")
                        nc.scalar.activation(
                            out=pt[:, r:CHUNK], in_=sc[:, r:CHUNK], func=Exp, scale=SCALE
                        )
                        if i >= 4 * j:
                            nc.vector.tensor_mul(
                                out=pt,
                                in0=pt,
                                in1=mask_t[:, 384 - rm : 384 - rm + CHUNK],
                            )
                        pts.append(pt)
                        if i % 2 == 1:
                            # pair-add on DVE halves the denominator matmuls
                            psum_t = pt_pool.tile(
                                [128, CHUNK], fp16, tag="pts", name="pts", bufs=18
                            )
                            nc.vector.tensor_add(
                                out=psum_t, in0=pts[i - 1], in1=pts[i]
                            )
                            psums.append(psum_t)
                    for i in range(n_t):
                        nc.tensor.matmul(
                            out=ctx,
                            lhsT=Vb[b][h][:, i * 128 : (i + 1) * 128],
                            rhs=pts[i],
                            start=(i == 0),
                            stop=(i == n_t - 1),
                        )
                    accs.append((ctx, psums))
                state[(b, j, "acc")] = accs

            def attn_b(b, j):
                accs = state.pop((b, j, "acc"))
                ctx_chunk = []
                n_t = 4 * j + 4
                for h in range(2):
                    ctx, psums = accs[h]
                    den = ps_misc.tile([128, CHUNK], fp32, tag="ps_misc", name="den")
                    for p2 in range(n_t // 2):
                        nc.tensor.matmul(
                            out=den,
                            lhsT=ones_t,
                            rhs=psums[p2],
                            start=(p2 == 0),
                            stop=(p2 == n_t // 2 - 1),
                        )
                    rec = rec_pool.tile([128, CHUNK], fp32, tag="rec", name="rec")
                    nc.vector.reciprocal(out=rec, in_=den)
                    cxt = cx_pool.tile([128, CHUNK], fp16, tag="cx", name="cx")
                    nc.vector.tensor_mul(out=cxt, in0=ctx, in1=rec)
                    ctx_chunk.append(cxt)
                state[(b, j)] = ctx_chunk

            def dense(b, j):
                tok0 = b * S + j * CHUNK
                ctx_chunk = state.pop((b, j))
                for mi in range(16):
                    po = ps_misc.tile([128, CHUNK], fp32, tag="ps_misc", name="po")
                    for h in range(2):
                        nc.tensor.matmul(
                            out=po,
                            lhsT=dw_all[:, h, mi * 128 : (mi + 1) * 128],
                            rhs=ctx_chunk[h],
                            start=(h == 0),
                            stop=(h == 1),
                        )
                    ot = ost_pool.tile([128, CHUNK], fp16, tag="ost", name="ot")
                    nc.vector.tensor_copy(out=ot, in_=po)
                    nc.sync.dma_start(
                        out=outT[mi * 128 : (mi + 1) * 128, tok0 : tok0 + CHUNK],
                        in_=ot,
                    )

            chunks = [(b, j) for b in range(B) for j in range(N_CHUNKS)]

            # interleave the first x chunk with the weight k-tiles so the
            # first accumulation isn't gated on the full weight DMA
            nc.sync.dma_start(out=qb_t, in_=qb.rearrange("(g p) -> p g", p=128))
            nc.sync.dma_start(out=mask_t, in_=mask)
            make_identity(nc, ident)
            nc.vector.memset(ones_t, 1.0)
            xk0 = []
            for k in range(K_TILES):
                xt = x_pool.tile([128, CHUNK], fp16, tag="xk", name="xk")
                nc.sync.dma_start(out=xt, in_=xT[k * 128 : (k + 1) * 128, 0:CHUNK])
                nc.sync.dma_start(out=w_all[:, k, :], in_=wq_v[:, k, :])
                if k in (7, 11):
                    nc.sync.dma_start(
                        out=dw_all[:, k // 4 - 1, :],
                        in_=dw.rearrange("(g p) o -> p g o", p=128)[:, k // 4 - 1, :],
                    )
                xk0.append(xt)

            # software pipeline: stage1 of the next chunk is emitted between
            # attn and dense of the current chunk so the PE always has
            # independent matmul work while the softmax chain drains
            stage1(*chunks[0], xk0)
            xk_next = load_x(*chunks[1])
            for ci, (b, j) in enumerate(chunks):
                attn_a(b, j)
                if ci + 1 < len(chunks):
                    stage1(*chunks[ci + 1], xk_next)
                attn_b(b, j)
                if ci + 2 < len(chunks):
                    xk_next = load_x(*chunks[ci + 2])
                dense(b, j)
    nc.compile()
    return nc


def _get_program():
    if "nc" not in _CACHE:
        _CACHE["nc"] = _build_program()
    return _CACHE["nc"]


def _host_inputs(hidden_states, qkv_w, qkv_b, dense_w):
    xT = (
        np.ascontiguousarray(
            hidden_states.astype(np.float16).transpose(2, 1, 0)
        ).reshape(H, B * S)
    )
    maskm = (
        np.arange(128)[:, None] <= (np.arange(896)[None, :] - 384)
    ).astype(np.float16)
    in_maps = []
    for c in range(NCORES):
        wqkvT = np.ascontiguousarray(
            qkv_w[c * 768 : (c + 1) * 768].astype(np.float16).T
        )
        qkvb = np.ascontiguousarray(qkv_b[c * 768 : (c + 1) * 768]).astype(np.float32)
        dwT = np.ascontiguousarray(
            dense_w[:, c * 256 : (c + 1) * 256].astype(np.float16).T
        )
        in_maps.append(
            {
                "xT": xT,
                "wqkvT": wqkvT,
                "qkvb": qkvb,
                "dwT": dwT,
                "maskm": maskm,
            }
        )
    return in_maps


def run_spmd(in_maps, **kwargs):
    from concourse import bass_utils

    nc = _get_program()
    return bass_utils.run_bass_kernel_spmd(
        nc, in_maps, core_ids=list(range(NCORES)), **kwargs
    )


def kernel(hidden_states, attention_mask, qkv_w, qkv_b, dense_w, dense_b):
    hidden_states = np.asarray(hidden_states, dtype=np.float32)
    qkv_w = np.asarray(qkv_w, dtype=np.float32)
    qkv_b = np.asarray(qkv_b, dtype=np.float32)
    dense_w = np.asarray(dense_w, dtype=np.float32)
    dense_b = np.asarray(dense_b, dtype=np.float32)

    in_maps = _host_inputs(hidden_states, qkv_w, qkv_b, dense_w)
    res = run_spmd(in_maps)
    acc = np.zeros((H, B * S), np.float32)
    for r in res.results:
        acc += r["outT"].astype(np.float32)
    out = acc.reshape(H, B, S).transpose(2, 1, 0)
    return np.ascontiguousarray(out), dense_b


# revision 10
# speedup vs baseline: 1.0113x; 1.0113x over previous
"""GPT3 parallel attention block on 8 Trainium2 NeuronCores.

Tensor-parallel over heads: each of the 8 cores owns 2 of the 16 heads.
Per core: QKV projection for its 768 channels, causal attention for its
2 heads x 2 batches, and the dense projection restricted to its head
columns, producing a partial [H, B*S] output. Partials are summed on the
host (the all-reduce of the reference sharding).

Layouts (device, per core):
  xT      [H, B*S]   fp16  hidden states transposed; token t = b*S + s
  wqkvT   [H, 768]   fp16  qkv weight slice, channels [q0 k0 v0 q1 k1 v1]
  qkvb    [768]      fp32  qkv bias slice (same channel order)
  dwT     [256, H]   fp16  dense weight slice, rows = (head, d) in-channels
  maskm   [128, 896] fp16  sliding causal mask master
  outT    [H, B*S]   fp16  partial output (out-channel major)

All matmuls run in fp16 operands / fp32 PSUM accumulation. Softmax is
unnormalized exp (no max subtraction; scores are O(1)) with the
denominator computed by an ones-matmul that replicates the row sum
across all 128 partitions, so the normalization is a plain elementwise
multiply by the DVE reciprocal.
"""

import math

import numpy as np

S, B, H, NH, D = 2048, 2, 2048, 16, 128
NCORES = 8
CHUNK = 512
N_CHUNKS = S // CHUNK  # 4
K_TILES = H // 128  # 16
SCALE = 1.0 / math.sqrt(float(D))  # coeff / (sqrt(d) * coeff)

_CACHE: dict = {}


def _build_program():
    import concourse.tile as tile
    from concourse import bacc, mybir
    from concourse.masks import make_identity

    fp16 = mybir.dt.float16
    fp32 = mybir.dt.float32

    nc = bacc.Bacc(
        "TRN2",
        target_bir_lowering=False,
        debug=False,
        enable_asserts=True,
        num_devices=NCORES,
    )
    xT = nc.dram_tensor("xT", [H, B * S], fp16, kind="ExternalInput").ap()
    wq = nc.dram_tensor("wqkvT", [H, 768], fp16, kind="ExternalInput").ap()
    qb = nc.dram_tensor("qkvb", [768], fp32, kind="ExternalInput").ap()
    dw = nc.dram_tensor("dwT", [256, H], fp16, kind="ExternalInput").ap()
    mask = nc.dram_tensor("maskm", [128, 896], fp16, kind="ExternalInput").ap()
    outT = nc.dram_tensor("outT", [H, B * S], fp16, kind="ExternalOutput").ap()

    with tile.TileContext(nc) as tc:
        with (
            tc.tile_pool(name="singles", bufs=1) as singles,
            tc.tile_pool(name="xk", bufs=36) as x_pool,
            tc.tile_pool(name="qt", bufs=4) as qt_pool,
            tc.tile_pool(name="kt", bufs=4) as kt_pool,
            tc.tile_pool(name="vv", bufs=4) as v_pool,
            tc.tile_pool(name="vt", bufs=3) as vt_pool,
            tc.tile_pool(name="pt", bufs=20) as pt_pool,
            tc.tile_pool(name="rec", bufs=2) as rec_pool,
            tc.tile_pool(name="cx", bufs=4) as cx_pool,
            tc.tile_pool(name="ost", bufs=6) as ost_pool,
            tc.tile_pool(name="ps_qkv", bufs=2, space="PSUM") as ps_qkv,
            tc.tile_pool(name="ps_misc", bufs=2, space="PSUM") as ps_misc,
            tc.tile_pool(name="ps_sc", bufs=2, space="PSUM") as ps_sc,
            tc.tile_pool(name="ps_ctx", bufs=2, space="PSUM") as ps_ctx,
        ):
            # --- one-time loads / constants (weight k-tiles split so the
            # first QKV accumulation can start before the full load lands)
            w_all = singles.tile([128, K_TILES, 768], fp16, tag="w_all")
            dw_all = singles.tile([128, 2, H], fp16, tag="dw_all")
            mask_t = singles.tile([128, 896], fp16, tag="mask_t")
            qb_t = singles.tile([128, 6], fp32, tag="qb_t")
            ident = singles.tile([128, 128], fp32, tag="ident")
            ones_t = singles.tile([128, 128], fp16, tag="ones_t")

            wq_v = wq.rearrange("(k p) c -> p k c", p=128)

            Ident = mybir.ActivationFunctionType.Identity
            Exp = mybir.ActivationFunctionType.Exp

            kT = {}
            Vb = {}
            qt = {}
            state = {}
            pt_gen = [0]  # first pass through the pt pool must write full tiles

            def load_x(b, j):
                tok0 = b * S + j * CHUNK
                xk = []
                for k in range(K_TILES):
                    xt = x_pool.tile([128, CHUNK], fp16, tag="xk", name="xk")
                    nc.sync.dma_start(
                        out=xt,
                        in_=xT[k * 128 : (k + 1) * 128, tok0 : tok0 + CHUNK],
                    )
                    xk.append(xt)
                return xk

            def stage1(b, j, xk):
                # q, k, v for chunk j of batch b, both heads, then V transpose
                if j == 0:
                    kT[b] = [
                        kt_pool.tile([128, S], fp16, tag="kt", name="kt")
                        for _ in range(2)
                    ]
                    Vb[b] = [
                        v_pool.tile([128, S], fp16, tag="vv", name="vv")
                        for _ in range(2)
                    ]
                vt = []
                qt[(b, j)] = []
                for h in range(2):
                    qtile = qt_pool.tile([128, CHUNK], fp16, tag="qt", name="qt")
                    vtile = vt_pool.tile([128, CHUNK], fp32, tag="vt", name="vt")
                    qt[(b, j)].append(qtile)
                    vt.append(vtile)
                    for which in range(3):  # q, k, v
                        ci = 3 * h + which
                        ps = ps_qkv.tile([128, CHUNK], fp32, tag="ps_qkv", name="ps")
                        for k in range(K_TILES):
                            nc.tensor.matmul(
                                out=ps,
                                lhsT=w_all[:, k, ci * 128 : (ci + 1) * 128],
                                rhs=xk[k],
                                start=(k == 0),
                                stop=(k == K_TILES - 1),
                            )
                        if which == 0:
                            dest = qtile
                        elif which == 1:
                            dest = kT[b][h][:, j * CHUNK : (j + 1) * CHUNK]
                        else:
                            dest = vtile
                        nc.vector.tensor_scalar_add(
                            out=dest, in0=ps, scalar1=qb_t[:, ci : ci + 1]
                        )
                for h in range(2):
                    tp = ps_misc.tile([128, CHUNK], fp32, tag="ps_misc", name="tp")
                    for ti in range(4):
                        nc.tensor.transpose(
                            out=tp[:, ti * 128 : (ti + 1) * 128],
                            in_=vt[h][:, ti * 128 : (ti + 1) * 128],
                            identity=ident,
                        )
                    nc.vector.tensor_copy(
                        out=Vb[b][h][:, j * CHUNK : (j + 1) * CHUNK], in_=tp
                    )

            def attn_a(b, j):
                accs = []
                for h in range(2):
                    n_t = 4 * j + 4
                    ctx = ps_ctx.tile([128, CHUNK], fp32, tag="ps_ctx", name="ctx")
                    pts = []
                    psums = []
                    for i in range(n_t):
                        # diagonal tiles only need columns >= r; the masked
                        # rest of pt is zeroed by the mask multiply. The first
                        # generation of each pool slot must be written fully
                        # (stale SBUF can hold inf/NaN bit patterns).
                        rm = (i - 4 * j) * 128 if i >= 4 * j else 0
                        r = 0 if pt_gen[0] < 20 else rm
                        pt_gen[0] += 1
                        sc = ps_sc.tile([128, CHUNK], fp32, tag="ps_sc", name="sc")
                        nc.tensor.matmul(
                            out=sc[:, r:CHUNK],
                            lhsT=kT[b][h][:, i * 128 : (i + 1) * 128],
                            rhs=qt[(b, j)][h][:, r:CHUNK],
                            start=True,
                            stop=True,
                        )
                        pt = pt_pool.tile([128, CHUNK], fp16, tag="pt", name="pt")
                        nc.scalar.activation(
                            out=pt[:, r:CHUNK], in_=sc[:, r:CHUNK], func=Exp, scale=SCALE
                        )
                        if i >= 4 * j:
                            nc.vector.tensor_mul(
                                out=pt,
                                in0=pt,
                                in1=mask_t[:, 384 - rm : 384 - rm + CHUNK],
                            )
                        pts.append(pt)
                        if i % 2 == 1:
                            # pair-add on DVE halves the denominator matmuls
                            psum_t = pt_pool.tile(
                                [128, CHUNK], fp16, tag="pts", name="pts", bufs=18
                            )
                            nc.vector.tensor_add(
                                out=psum_t, in0=pts[i - 1], in1=pts[i]
                            )
                            psums.append(psum_t)
                    for i in range(n_t):
                        nc.tensor.matmul(
                            out=ctx,
                            lhsT=Vb[b][h][:, i * 128 : (i + 1) * 128],
                            rhs=pts[i],
                            start=(i == 0),
                            stop=(i == n_t - 1),
                        )
                    accs.append((ctx, psums))
                state[(b, j, "acc")] = accs

            def attn_b(b, j):
                accs = state.pop((b, j, "acc"))
                ctx_chunk = []
                n_t = 4 * j + 4
                for h in range(2):
                    ctx, psums = accs[h]
                    den = ps_qkv.tile([128, CHUNK], fp32, tag="ps_qkv", name="den")
                    for p2 in range(n_t // 2):
                        nc.tensor.matmul(
                            out=den,
                            lhsT=ones_t,
                            rhs=psums[p2],
                            start=(p2 == 0),
                            stop=(p2 == n_t // 2 - 1),
                        )
                    rec = rec_pool.tile([128, CHUNK], fp32, tag="rec", name="rec")
                    nc.vector.reciprocal(out=rec, in_=den)
                    cxt = cx_pool.tile([128, CHUNK], fp16, tag="cx", name="cx")
                    nc.vector.tensor_mul(out=cxt, in0=ctx, in1=rec)
                    ctx_chunk.append(cxt)
                state[(b, j)] = ctx_chunk

            def dense(b, j):
                tok0 = b * S + j * CHUNK
                ctx_chunk = state.pop((b, j))
                for mi in range(16):
                    po = ps_misc.tile([128, CHUNK], fp32, tag="ps_misc", name="po")
                    for h in range(2):
                        nc.tensor.matmul(
                            out=po,
                            lhsT=dw_all[:, h, mi * 128 : (mi + 1) * 128],
                            rhs=ctx_chunk[h],
                            start=(h == 0),
                            stop=(h == 1),
                        )
                    ot = ost_pool.tile([128, CHUNK], fp16, tag="ost", name="ot")
                    nc.vector.tensor_copy(out=ot, in_=po)
                    nc.sync.dma_start(
                        out=outT[mi * 128 : (mi + 1) * 128, tok0 : tok0 + CHUNK],
                        in_=ot,
                    )

            chunks = [(b, j) for b in range(B) for j in range(N_CHUNKS)]

            # interleave the first x chunk with the weight k-tiles so the
            # first accumulation isn't gated on the full weight DMA
            nc.sync.dma_start(out=qb_t, in_=qb.rearrange("(g p) -> p g", p=128))
            nc.sync.dma_start(out=mask_t, in_=mask)
            make_identity(nc, ident)
            nc.vector.memset(ones_t, 1.0)
            xk0 = []
            for k in range(K_TILES):
                xt = x_pool.tile([128, CHUNK], fp16, tag="xk", name="xk")
                nc.sync.dma_start(out=xt, in_=xT[k * 128 : (k + 1) * 128, 0:CHUNK])
                nc.sync.dma_start(out=w_all[:, k, :], in_=wq_v[:, k, :])
                if k in (7, 11):
                    nc.sync.dma_start(
                        out=dw_all[:, k // 4 - 1, :],
                        in_=dw.rearrange("(g p) o -> p g o", p=128)[:, k // 4 - 1, :],
                    )
                xk0.append(xt)

            # software pipeline: stage1 of the next chunk is emitted between
            # attn and dense of the current chunk so the PE always has
            # independent matmul work while the softmax chain drains
            stage1(*chunks[0], xk0)
            xk_next = load_x(*chunks[1])
            for ci, (b, j) in enumerate(chunks):
                attn_a(b, j)
                if ci + 1 < len(chunks):
                    stage1(*chunks[ci + 1], xk_next)
                attn_b(b, j)
                if ci + 2 < len(chunks):
                    xk_next = load_x(*chunks[ci + 2])
                dense(b, j)
    nc.compile()
    return nc


def _get_program():
    if "nc" not in _CACHE:
        _CACHE["nc"] = _build_program()
    return _CACHE["nc"]


def _host_inputs(hidden_states, qkv_w, qkv_b, dense_w):
    xT = (
        np.ascontiguousarray(
            hidden_states.astype(np.float16).transpose(2, 1, 0)
        ).reshape(H, B * S)
    )
    maskm = (
        np.arange(128)[:, None] <= (np.arange(896)[None, :] - 384)
    ).astype(np.float16)
    in_maps = []
    for c in range(NCORES):
        wqkvT = np.ascontiguousarray(
            qkv_w[c * 768 : (c + 1) * 768].astype(np.float16).T
        )
        qkvb = np.ascontiguousarray(qkv_b[c * 768 : (c + 1) * 768]).astype(np.float32)
        dwT = np.ascontiguousarray(
            dense_w[:, c * 256 : (c + 1) * 256].astype(np.float16).T
        )
        in_maps.append(
            {
                "xT": xT,
                "wqkvT": wqkvT,
                "qkvb": qkvb,
                "dwT": dwT,
                "maskm": maskm,
            }
        )
    return in_maps


def run_spmd(in_maps, **kwargs):
    from concourse import bass_utils

    nc = _get_program()
    return bass_utils.run_bass_kernel_spmd(
        nc, in_maps, core_ids=list(range(NCORES)), **kwargs
    )


def kernel(hidden_states, attention_mask, qkv_w, qkv_b, dense_w, dense_b):
    hidden_states = np.asarray(hidden_states, dtype=np.float32)
    qkv_w = np.asarray(qkv_w, dtype=np.float32)
    qkv_b = np.asarray(qkv_b, dtype=np.float32)
    dense_w = np.asarray(dense_w, dtype=np.float32)
    dense_b = np.asarray(dense_b, dtype=np.float32)

    in_maps = _host_inputs(hidden_states, qkv_w, qkv_b, dense_w)
    res = run_spmd(in_maps)
    acc = np.zeros((H, B * S), np.float32)
    for r in res.results:
        acc += r["outT"].astype(np.float32)
    out = acc.reshape(H, B, S).transpose(2, 1, 0)
    return np.ascontiguousarray(out), dense_b


# revision 11
# speedup vs baseline: 1.1442x; 1.1314x over previous
"""GPT3 parallel attention block on 8 Trainium2 NeuronCores.

Tensor-parallel over heads: each of the 8 cores owns 2 of the 16 heads.
Per core: QKV projection for its 768 channels, causal attention for its
2 heads x 2 batches, and the dense projection restricted to its head
columns, producing a partial [H, B*S] output. Partials are summed on the
host (the all-reduce of the reference sharding).

Layouts (device, per core):
  xT      [H, B*S]   fp16  hidden states transposed; token t = b*S + s
  wqkvT   [H, 768]   fp16  qkv weight slice, channels [q0 k0 v0 q1 k1 v1]
  qkvb    [768]      fp32  qkv bias slice (same channel order)
  dwT     [256, H]   fp16  dense weight slice, rows = (head, d) in-channels
  maskm   [128, 896] fp16  sliding causal mask master
  outT    [H, B*S]   fp16  partial output (out-channel major)

All matmuls run in fp16 operands / fp32 PSUM accumulation. Softmax is
unnormalized exp (no max subtraction; scores are O(1)) with the
denominator computed by an ones-matmul that replicates the row sum
across all 128 partitions, so the normalization is a plain elementwise
multiply by the DVE reciprocal.
"""

import math

import numpy as np

S, B, H, NH, D = 2048, 2, 2048, 16, 128
NCORES = 8
CHUNK = 512
N_CHUNKS = S // CHUNK  # 4
K_TILES = H // 128  # 16
SCALE = 1.0 / math.sqrt(float(D))  # coeff / (sqrt(d) * coeff)

_CACHE: dict = {}


def _build_program():
    import concourse.tile as tile
    from concourse import bacc, mybir
    from concourse.masks import make_identity

    fp16 = mybir.dt.float16
    fp32 = mybir.dt.float32

    nc = bacc.Bacc(
        "TRN2",
        target_bir_lowering=False,
        debug=False,
        enable_asserts=True,
        num_devices=NCORES,
    )
    xT = nc.dram_tensor("xT", [H, B * S], fp16, kind="ExternalInput").ap()
    wq = nc.dram_tensor("wqkvT", [H, 768], fp16, kind="ExternalInput").ap()
    qb = nc.dram_tensor("qkvb", [768], fp32, kind="ExternalInput").ap()
    dw = nc.dram_tensor("dwT", [256, H], fp16, kind="ExternalInput").ap()
    mask = nc.dram_tensor("maskm", [128, 896], fp16, kind="ExternalInput").ap()
    outT = nc.dram_tensor("outT", [H, B * S], fp16, kind="ExternalOutput").ap()

    with tile.TileContext(nc) as tc:
        with (
            tc.tile_pool(name="singles", bufs=1) as singles,
            tc.tile_pool(name="xk", bufs=36) as x_pool,
            tc.tile_pool(name="qt", bufs=4) as qt_pool,
            tc.tile_pool(name="kt", bufs=4) as kt_pool,
            tc.tile_pool(name="vv", bufs=4) as v_pool,
            tc.tile_pool(name="vt", bufs=3) as vt_pool,
            tc.tile_pool(name="pt", bufs=20) as pt_pool,
            tc.tile_pool(name="rec", bufs=2) as rec_pool,
            tc.tile_pool(name="cx", bufs=4) as cx_pool,
            tc.tile_pool(name="ost", bufs=6) as ost_pool,
            tc.tile_pool(name="ps_qkv", bufs=2, space="PSUM") as ps_qkv,
            tc.tile_pool(name="ps_misc", bufs=2, space="PSUM") as ps_misc,
            tc.tile_pool(name="ps_sc", bufs=2, space="PSUM") as ps_sc,
            tc.tile_pool(name="ps_ctx", bufs=2, space="PSUM") as ps_ctx,
        ):
            # --- one-time loads / constants (weight k-tiles split so the
            # first QKV accumulation can start before the full load lands)
            w_all = singles.tile([128, K_TILES, 768], fp16, tag="w_all")
            dw_all = singles.tile([128, 2, H], fp16, tag="dw_all")
            mask_t = singles.tile([128, 896], fp16, tag="mask_t")
            qb_t = singles.tile([128, 6], fp32, tag="qb_t")
            ident = singles.tile([128, 128], fp32, tag="ident")
            ones_t = singles.tile([128, 128], fp16, tag="ones_t")

            wq_v = wq.rearrange("(k p) c -> p k c", p=128)

            Ident = mybir.ActivationFunctionType.Identity
            Exp = mybir.ActivationFunctionType.Exp

            kT = {}
            Vb = {}
            qt = {}
            state = {}
            pt_gen = [0]  # first pass through the pt pool must write full tiles

            def load_x(b, j):
                tok0 = b * S + j * CHUNK
                xk = []
                for k in range(K_TILES):
                    xt = x_pool.tile([128, CHUNK], fp16, tag="xk", name="xk")
                    nc.sync.dma_start(
                        out=xt,
                        in_=xT[k * 128 : (k + 1) * 128, tok0 : tok0 + CHUNK],
                    )
                    xk.append(xt)
                return xk

            def stage1(b, j, xk):
                # q, k, v for chunk j of batch b, both heads, then V transpose
                if j == 0:
                    kT[b] = [
                        kt_pool.tile([128, S], fp16, tag="kt", name="kt")
                        for _ in range(2)
                    ]
                    Vb[b] = [
                        v_pool.tile([128, S], fp16, tag="vv", name="vv")
                        for _ in range(2)
                    ]
                vt = []
                qt[(b, j)] = []
                for h in range(2):
                    qtile = qt_pool.tile([128, CHUNK], fp16, tag="qt", name="qt")
                    vtile = vt_pool.tile([128, CHUNK], fp32, tag="vt", name="vt")
                    qt[(b, j)].append(qtile)
                    vt.append(vtile)
                    for which in range(3):  # q, k, v
                        ci = 3 * h + which
                        ps = ps_qkv.tile([128, CHUNK], fp32, tag="ps_qkv", name="ps")
                        for k in range(K_TILES):
                            nc.tensor.matmul(
                                out=ps,
                                lhsT=w_all[:, k, ci * 128 : (ci + 1) * 128],
                                rhs=xk[k],
                                start=(k == 0),
                                stop=(k == K_TILES - 1),
                            )
                        if which == 0:
                            dest = qtile
                        elif which == 1:
                            dest = kT[b][h][:, j * CHUNK : (j + 1) * CHUNK]
                        else:
                            dest = vtile
                        nc.vector.tensor_scalar_add(
                            out=dest, in0=ps, scalar1=qb_t[:, ci : ci + 1]
                        )
                for h in range(2):
                    tp = ps_misc.tile([128, CHUNK], fp32, tag="ps_misc", name="tp")
                    for ti in range(4):
                        nc.tensor.transpose(
                            out=tp[:, ti * 128 : (ti + 1) * 128],
                            in_=vt[h][:, ti * 128 : (ti + 1) * 128],
                            identity=ident,
                        )
                    nc.vector.tensor_copy(
                        out=Vb[b][h][:, j * CHUNK : (j + 1) * CHUNK], in_=tp
                    )

            def attn_a(b, j):
                accs = []
                for h in range(2):
                    n_t = 4 * j + 4
                    ctx = ps_ctx.tile([128, CHUNK], fp32, tag="ps_ctx", name="ctx")
                    pts = []
                    psums = []
                    for i in range(n_t):
                        # diagonal tiles only need columns >= r; the masked
                        # rest of pt is zeroed by the mask multiply. The first
                        # generation of each pool slot must be written fully
                        # (stale SBUF can hold inf/NaN bit patterns).
                        rm = (i - 4 * j) * 128 if i >= 4 * j else 0
                        r = 0 if pt_gen[0] < 20 else rm
                        pt_gen[0] += 1
                        sc = ps_sc.tile([128, CHUNK], fp32, tag="ps_sc", name="sc")
                        nc.tensor.matmul(
                            out=sc[:, r:CHUNK],
                            lhsT=kT[b][h][:, i * 128 : (i + 1) * 128],
                            rhs=qt[(b, j)][h][:, r:CHUNK],
                            start=True,
                            stop=True,
                        )
                        pt = pt_pool.tile([128, CHUNK], fp16, tag="pt", name="pt")
                        nc.scalar.activation(
                            out=pt[:, r:CHUNK], in_=sc[:, r:CHUNK], func=Exp, scale=SCALE
                        )
                        if i >= 4 * j:
                            nc.vector.tensor_mul(
                                out=pt,
                                in0=pt,
                                in1=mask_t[:, 384 - rm : 384 - rm + CHUNK],
                            )
                        pts.append(pt)
                        if i % 2 == 1:
                            # pair-add on DVE halves the denominator matmuls
                            psum_t = pt_pool.tile(
                                [128, CHUNK], fp16, tag="pts", name="pts", bufs=18
                            )
                            nc.vector.tensor_add(
                                out=psum_t, in0=pts[i - 1], in1=pts[i]
                            )
                            psums.append(psum_t)
                    for i in range(n_t):
                        nc.tensor.matmul(
                            out=ctx,
                            lhsT=Vb[b][h][:, i * 128 : (i + 1) * 128],
                            rhs=pts[i],
                            start=(i == 0),
                            stop=(i == n_t - 1),
                        )
                    accs.append((ctx, psums))
                state[(b, j, "acc")] = accs

            def attn_b(b, j):
                accs = state.pop((b, j, "acc"))
                ctx_chunk = []
                n_t = 4 * j + 4
                for h in range(2):
                    ctx, psums = accs[h]
                    den = ps_qkv.tile([128, CHUNK], fp32, tag="ps_qkv", name="den")
                    for p2 in range(n_t // 2):
                        nc.tensor.matmul(
                            out=den,
                            lhsT=ones_t,
                            rhs=psums[p2],
                            start=(p2 == 0),
                            stop=(p2 == n_t // 2 - 1),
                        )
                    rec = rec_pool.tile([128, CHUNK], fp32, tag="rec", name="rec")
                    nc.vector.reciprocal_approx_fast(out=rec, in_=den)
                    cxt = cx_pool.tile([128, CHUNK], fp16, tag="cx", name="cx")
                    nc.vector.tensor_mul(out=cxt, in0=ctx, in1=rec)
                    ctx_chunk.append(cxt)
                state[(b, j)] = ctx_chunk

            def dense(b, j):
                tok0 = b * S + j * CHUNK
                ctx_chunk = state.pop((b, j))
                for mi in range(16):
                    po = ps_misc.tile([128, CHUNK], fp32, tag="ps_misc", name="po")
                    for h in range(2):
                        nc.tensor.matmul(
                            out=po,
                            lhsT=dw_all[:, h, mi * 128 : (mi + 1) * 128],
                            rhs=ctx_chunk[h],
                            start=(h == 0),
                            stop=(h == 1),
                        )
                    ot = ost_pool.tile([128, CHUNK], fp16, tag="ost", name="ot")
                    nc.vector.tensor_copy(out=ot, in_=po)
                    nc.sync.dma_start(
                        out=outT[mi * 128 : (mi + 1) * 128, tok0 : tok0 + CHUNK],
                        in_=ot,
                    )

            chunks = [(b, j) for b in range(B) for j in range(N_CHUNKS)]

            # interleave the first x chunk with the weight k-tiles so the
            # first accumulation isn't gated on the full weight DMA
            nc.sync.dma_start(out=qb_t, in_=qb.rearrange("(g p) -> p g", p=128))
            nc.sync.dma_start(out=mask_t, in_=mask)
            make_identity(nc, ident)
            nc.vector.memset(ones_t, 1.0)
            xk0 = []
            for k in range(K_TILES):
                xt = x_pool.tile([128, CHUNK], fp16, tag="xk", name="xk")
                nc.sync.dma_start(out=xt, in_=xT[k * 128 : (k + 1) * 128, 0:CHUNK])
                nc.sync.dma_start(out=w_all[:, k, :], in_=wq_v[:, k, :])
                if k in (7, 11):
                    nc.sync.dma_start(
                        out=dw_all[:, k // 4 - 1, :],
                        in_=dw.rearrange("(g p) o -> p g o", p=128)[:, k // 4 - 1, :],
                    )
                xk0.append(xt)

            # software pipeline: stage1 of the next chunk is emitted between
            # attn and dense of the current chunk so the PE always has
            # independent matmul work while the softmax chain drains
            stage1(*chunks[0], xk0)
            xk_next = load_x(*chunks[1])
            for ci, (b, j) in enumerate(chunks):
                attn_a(b, j)
                if ci + 1 < len(chunks):
                    stage1(*chunks[ci + 1], xk_next)
                attn_b(b, j)
                if ci + 2 < len(chunks):
                    xk_next = load_x(*chunks[ci + 2])
                dense(b, j)
    nc.compile()
    return nc


def _get_program():
    if "nc" not in _CACHE:
        _CACHE["nc"] = _build_program()
    return _CACHE["nc"]


def _host_inputs(hidden_states, qkv_w, qkv_b, dense_w):
    xT = (
        np.ascontiguousarray(
            hidden_states.astype(np.float16).transpose(2, 1, 0)
        ).reshape(H, B * S)
    )
    maskm = (
        np.arange(128)[:, None] <= (np.arange(896)[None, :] - 384)
    ).astype(np.float16)
    in_maps = []
    for c in range(NCORES):
        wqkvT = np.ascontiguousarray(
            qkv_w[c * 768 : (c + 1) * 768].astype(np.float16).T
        )
        qkvb = np.ascontiguousarray(qkv_b[c * 768 : (c + 1) * 768]).astype(np.float32)
        dwT = np.ascontiguousarray(
            dense_w[:, c * 256 : (c + 1) * 256].astype(np.float16).T
        )
        in_maps.append(
            {
                "xT": xT,
                "wqkvT": wqkvT,
                "qkvb": qkvb,
                "dwT": dwT,
                "maskm": maskm,
            }
        )
    return in_maps


def run_spmd(in_maps, **kwargs):
    from concourse import bass_utils

    nc = _get_program()
    return bass_utils.run_bass_kernel_spmd(
        nc, in_maps, core_ids=list(range(NCORES)), **kwargs
    )


def kernel(hidden_states, attention_mask, qkv_w, qkv_b, dense_w, dense_b):
    hidden_states = np.asarray(hidden_states, dtype=np.float32)
    qkv_w = np.asarray(qkv_w, dtype=np.float32)
    qkv_b = np.asarray(qkv_b, dtype=np.float32)
    dense_w = np.asarray(dense_w, dtype=np.float32)
    dense_b = np.asarray(dense_b, dtype=np.float32)

    in_maps = _host_inputs(hidden_states, qkv_w, qkv_b, dense_w)
    res = run_spmd(in_maps)
    acc = np.zeros((H, B * S), np.float32)
    for r in res.results:
        acc += r["outT"].astype(np.float32)
    out = acc.reshape(H, B, S).transpose(2, 1, 0)
    return np.ascontiguousarray(out), dense_b


# revision 13
# speedup vs baseline: 1.1763x; 1.0280x over previous
"""GPT3 parallel attention block on 8 Trainium2 NeuronCores.

Tensor-parallel over heads: each of the 8 cores owns 2 of the 16 heads.
Per core: QKV projection for its 768 channels, causal attention for its
2 heads x 2 batches, and the dense projection restricted to its head
columns, producing a partial [H, B*S] output. Partials are summed on the
host (the all-reduce of the reference sharding).

Layouts (device, per core):
  xT      [H, B*S]   fp16  hidden states transposed; token t = b*S + s
  wqkvT   [H, 768]   fp16  qkv weight slice, channels [q0 k0 v0 q1 k1 v1]
  qkvb    [768]      fp32  qkv bias slice (same channel order)
  dwT     [256, H]   fp16  dense weight slice, rows = (head, d) in-channels
  maskm   [128, 896] fp16  sliding causal mask master
  outT    [H, B*S]   fp16  partial output (out-channel major)

All matmuls run in fp16 operands / fp32 PSUM accumulation. Softmax is
unnormalized exp (no max subtraction; scores are O(1)) with the
denominator computed by an ones-matmul that replicates the row sum
across all 128 partitions, so the normalization is a plain elementwise
multiply by the DVE reciprocal.
"""

import math

import numpy as np

S, B, H, NH, D = 2048, 2, 2048, 16, 128
NCORES = 8
CHUNK = 512
N_CHUNKS = S // CHUNK  # 4
K_TILES = H // 128  # 16
SCALE = 1.0 / math.sqrt(float(D))  # coeff / (sqrt(d) * coeff)

_CACHE: dict = {}


def _build_program():
    import concourse.tile as tile
    from concourse import bacc, mybir
    from concourse.masks import make_identity

    fp16 = mybir.dt.float16
    fp32 = mybir.dt.float32

    nc = bacc.Bacc(
        "TRN2",
        target_bir_lowering=False,
        debug=False,
        enable_asserts=True,
        num_devices=NCORES,
    )
    xT = nc.dram_tensor("xT", [H, B * S], fp16, kind="ExternalInput").ap()
    wq = nc.dram_tensor("wqkvT", [H, 768], fp16, kind="ExternalInput").ap()
    qb = nc.dram_tensor("qkvb", [768], fp32, kind="ExternalInput").ap()
    dw = nc.dram_tensor("dwT", [256, H], fp16, kind="ExternalInput").ap()
    mask = nc.dram_tensor("maskm", [128, 896], fp16, kind="ExternalInput").ap()
    outT = nc.dram_tensor("outT", [H, B * S], fp16, kind="ExternalOutput").ap()

    with tile.TileContext(nc) as tc:
        with (
            tc.tile_pool(name="singles", bufs=1) as singles,
            tc.tile_pool(name="xk", bufs=36) as x_pool,
            tc.tile_pool(name="qt", bufs=4) as qt_pool,
            tc.tile_pool(name="kt", bufs=4) as kt_pool,
            tc.tile_pool(name="vv", bufs=4) as v_pool,
            tc.tile_pool(name="vt", bufs=3) as vt_pool,
            tc.tile_pool(name="pt", bufs=20) as pt_pool,
            tc.tile_pool(name="rec", bufs=2) as rec_pool,
            tc.tile_pool(name="cx", bufs=4) as cx_pool,
            tc.tile_pool(name="ost", bufs=6) as ost_pool,
            tc.tile_pool(name="ps_qkv", bufs=2, space="PSUM") as ps_qkv,
            tc.tile_pool(name="ps_misc", bufs=2, space="PSUM") as ps_misc,
            tc.tile_pool(name="ps_sc", bufs=2, space="PSUM") as ps_sc,
            tc.tile_pool(name="ps_ctx", bufs=2, space="PSUM") as ps_ctx,
        ):
            # --- one-time loads / constants (weight k-tiles split so the
            # first QKV accumulation can start before the full load lands)
            w_all = singles.tile([128, K_TILES, 768], fp16, tag="w_all")
            dw_all = singles.tile([128, 2, H], fp16, tag="dw_all")
            mask_t = singles.tile([128, 896], fp16, tag="mask_t")
            qb_t = singles.tile([128, 6], fp32, tag="qb_t")
            ident = singles.tile([128, 128], fp16, tag="ident")
            ones_t = singles.tile([128, 128], fp16, tag="ones_t")

            wq_v = wq.rearrange("(k p) c -> p k c", p=128)

            Ident = mybir.ActivationFunctionType.Identity
            Exp = mybir.ActivationFunctionType.Exp

            kT = {}
            Vb = {}
            qt = {}
            state = {}
            pt_gen = [0]  # first pass through the pt pool must write full tiles

            def load_x(b, j):
                tok0 = b * S + j * CHUNK
                xk = []
                for k in range(K_TILES):
                    xt = x_pool.tile([128, CHUNK], fp16, tag="xk", name="xk")
                    nc.sync.dma_start(
                        out=xt,
                        in_=xT[k * 128 : (k + 1) * 128, tok0 : tok0 + CHUNK],
                    )
                    xk.append(xt)
                return xk

            def stage1(b, j, xk):
                # q, k, v for chunk j of batch b, both heads, then V transpose
                if j == 0:
                    kT[b] = [
                        kt_pool.tile([128, S], fp16, tag="kt", name="kt")
                        for _ in range(2)
                    ]
                    Vb[b] = [
                        v_pool.tile([128, S], fp16, tag="vv", name="vv")
                        for _ in range(2)
                    ]
                vt = []
                qt[(b, j)] = []
                for h in range(2):
                    qtile = qt_pool.tile([128, CHUNK], fp16, tag="qt", name="qt")
                    vtile = vt_pool.tile([128, CHUNK], fp16, tag="vt", name="vt")
                    qt[(b, j)].append(qtile)
                    vt.append(vtile)
                    for which in range(3):  # q, k, v
                        ci = 3 * h + which
                        ps = ps_qkv.tile([128, CHUNK], fp32, tag="ps_qkv", name="ps")
                        for k in range(K_TILES):
                            nc.tensor.matmul(
                                out=ps,
                                lhsT=w_all[:, k, ci * 128 : (ci + 1) * 128],
                                rhs=xk[k],
                                start=(k == 0),
                                stop=(k == K_TILES - 1),
                            )
                        if which == 0:
                            dest = qtile
                        elif which == 1:
                            dest = kT[b][h][:, j * CHUNK : (j + 1) * CHUNK]
                        else:
                            dest = vtile
                        nc.vector.tensor_scalar_add(
                            out=dest, in0=ps, scalar1=qb_t[:, ci : ci + 1]
                        )
                for h in range(2):
                    tp = ps_misc.tile([128, CHUNK], fp16, tag="ps_misc", name="tp")
                    for ti in range(4):
                        nc.tensor.transpose(
                            out=tp[:, ti * 128 : (ti + 1) * 128],
                            in_=vt[h][:, ti * 128 : (ti + 1) * 128],
                            identity=ident,
                        )
                    nc.vector.tensor_copy(
                        out=Vb[b][h][:, j * CHUNK : (j + 1) * CHUNK], in_=tp
                    )

            def attn_a(b, j):
                accs = []
                for h in range(2):
                    n_t = 4 * j + 4
                    ctx = ps_ctx.tile([128, CHUNK], fp32, tag="ps_ctx", name="ctx")
                    pts = []
                    psums = []
                    quads = []
                    for i in range(n_t):
                        # diagonal tiles only need columns >= r; the masked
                        # rest of pt is zeroed by the mask multiply. The first
                        # generation of each pool slot must be written fully
                        # (stale SBUF can hold inf/NaN bit patterns).
                        rm = (i - 4 * j) * 128 if i >= 4 * j else 0
                        r = 0 if pt_gen[0] < 20 else rm
                        pt_gen[0] += 1
                        sc = ps_sc.tile([128, CHUNK], fp32, tag="ps_sc", name="sc")
                        nc.tensor.matmul(
                            out=sc[:, r:CHUNK],
                            lhsT=kT[b][h][:, i * 128 : (i + 1) * 128],
                            rhs=qt[(b, j)][h][:, r:CHUNK],
                            start=True,
                            stop=True,
                        )
                        pt = pt_pool.tile([128, CHUNK], fp16, tag="pt", name="pt")
                        nc.scalar.activation(
                            out=pt[:, r:CHUNK], in_=sc[:, r:CHUNK], func=Exp, scale=SCALE
                        )
                        if i >= 4 * j:
                            nc.vector.tensor_mul(
                                out=pt,
                                in0=pt,
                                in1=mask_t[:, 384 - rm : 384 - rm + CHUNK],
                            )
                        pts.append(pt)
                        if i % 2 == 1:
                            # pair- then quad-add on DVE: 4x fewer den matmuls
                            psum_t = pt_pool.tile(
                                [128, CHUNK], fp16, tag="pts", name="pts", bufs=4
                            )
                            nc.vector.tensor_add(
                                out=psum_t, in0=pts[i - 1], in1=pts[i]
                            )
                            psums.append(psum_t)
                            if i % 4 == 3:
                                q_t = pt_pool.tile(
                                    [128, CHUNK], fp16, tag="ptq", name="ptq", bufs=10
                                )
                                nc.vector.tensor_add(
                                    out=q_t, in0=psums[-2], in1=psums[-1]
                                )
                                quads.append(q_t)
                    for i in range(n_t):
                        nc.tensor.matmul(
                            out=ctx,
                            lhsT=Vb[b][h][:, i * 128 : (i + 1) * 128],
                            rhs=pts[i],
                            start=(i == 0),
                            stop=(i == n_t - 1),
                        )
                    accs.append((ctx, quads))
                state[(b, j, "acc")] = accs

            def attn_b(b, j):
                accs = state.pop((b, j, "acc"))
                ctx_chunk = []
                n_t = 4 * j + 4
                for h in range(2):
                    ctx, quads = accs[h]
                    den = ps_qkv.tile([128, CHUNK], fp32, tag="ps_qkv", name="den")
                    for p4 in range(n_t // 4):
                        nc.tensor.matmul(
                            out=den,
                            lhsT=ones_t,
                            rhs=quads[p4],
                            start=(p4 == 0),
                            stop=(p4 == n_t // 4 - 1),
                        )
                    rec = rec_pool.tile([128, CHUNK], fp32, tag="rec", name="rec")
                    nc.vector.reciprocal_approx_fast(out=rec, in_=den)
                    cxt = cx_pool.tile([128, CHUNK], fp16, tag="cx", name="cx")
                    nc.vector.tensor_mul(out=cxt, in0=ctx, in1=rec)
                    ctx_chunk.append(cxt)
                state[(b, j)] = ctx_chunk

            def dense(b, j):
                tok0 = b * S + j * CHUNK
                ctx_chunk = state.pop((b, j))
                for mi in range(16):
                    po = ps_misc.tile([128, CHUNK], fp32, tag="ps_misc", name="po")
                    for h in range(2):
                        nc.tensor.matmul(
                            out=po,
                            lhsT=dw_all[:, h, mi * 128 : (mi + 1) * 128],
                            rhs=ctx_chunk[h],
                            start=(h == 0),
                            stop=(h == 1),
                        )
                    ot = ost_pool.tile([128, CHUNK], fp16, tag="ost", name="ot")
                    nc.vector.tensor_copy(out=ot, in_=po)
                    nc.sync.dma_start(
                        out=outT[mi * 128 : (mi + 1) * 128, tok0 : tok0 + CHUNK],
                        in_=ot,
                    )

            chunks = [(b, j) for b in range(B) for j in range(N_CHUNKS)]

            # interleave the first x chunk with the weight k-tiles so the
            # first accumulation isn't gated on the full weight DMA
            nc.sync.dma_start(out=qb_t, in_=qb.rearrange("(g p) -> p g", p=128))
            nc.sync.dma_start(out=mask_t, in_=mask)
            make_identity(nc, ident)
            nc.vector.memset(ones_t, 1.0)
            xk0 = []
            for k in range(K_TILES):
                xt = x_pool.tile([128, CHUNK], fp16, tag="xk", name="xk")
                nc.sync.dma_start(out=xt, in_=xT[k * 128 : (k + 1) * 128, 0:CHUNK])
                nc.sync.dma_start(out=w_all[:, k, :], in_=wq_v[:, k, :])
                xk0.append(xt)
            nc.sync.dma_start(out=dw_all, in_=dw.rearrange("(g p) o -> p g o", p=128))

            # software pipeline: stage1 of the next chunk is emitted between
            # attn and dense of the current chunk so the PE always has
            # independent matmul work while the softmax chain drains
            stage1(*chunks[0], xk0)
            xk_next = load_x(*chunks[1])
            for ci, (b, j) in enumerate(chunks):
                attn_a(b, j)
                if ci + 1 < len(chunks):
                    stage1(*chunks[ci + 1], xk_next)
                attn_b(b, j)
                if ci + 2 < len(chunks):
                    xk_next = load_x(*chunks[ci + 2])
                dense(b, j)
    nc.compile()
    return nc


def _get_program():
    if "nc" not in _CACHE:
        _CACHE["nc"] = _build_program()
    return _CACHE["nc"]


def _host_inputs(hidden_states, qkv_w, qkv_b, dense_w):
    xT = (
        np.ascontiguousarray(
            hidden_states.astype(np.float16).transpose(2, 1, 0)
        ).reshape(H, B * S)
    )
    maskm = (
        np.arange(128)[:, None] <= (np.arange(896)[None, :] - 384)
    ).astype(np.float16)
    in_maps = []
    for c in range(NCORES):
        wqkvT = np.ascontiguousarray(
            qkv_w[c * 768 : (c + 1) * 768].astype(np.float16).T
        )
        qkvb = np.ascontiguousarray(qkv_b[c * 768 : (c + 1) * 768]).astype(np.float32)
        dwT = np.ascontiguousarray(
            dense_w[:, c * 256 : (c + 1) * 256].astype(np.float16).T
        )
        in_maps.append(
            {
                "xT": xT,
                "wqkvT": wqkvT,
                "qkvb": qkvb,
                "dwT": dwT,
                "maskm": maskm,
            }
        )
    return in_maps


def run_spmd(in_maps, **kwargs):
    from concourse import bass_utils

    nc = _get_program()
    return bass_utils.run_bass_kernel_spmd(
        nc, in_maps, core_ids=list(range(NCORES)), **kwargs
    )


def kernel(hidden_states, attention_mask, qkv_w, qkv_b, dense_w, dense_b):
    hidden_states = np.asarray(hidden_states, dtype=np.float32)
    qkv_w = np.asarray(qkv_w, dtype=np.float32)
    qkv_b = np.asarray(qkv_b, dtype=np.float32)
    dense_w = np.asarray(dense_w, dtype=np.float32)
    dense_b = np.asarray(dense_b, dtype=np.float32)

    in_maps = _host_inputs(hidden_states, qkv_w, qkv_b, dense_w)
    res = run_spmd(in_maps)
    acc = np.zeros((H, B * S), np.float32)
    for r in res.results:
        acc += r["outT"].astype(np.float32)
    out = acc.reshape(H, B, S).transpose(2, 1, 0)
    return np.ascontiguousarray(out), dense_b


# revision 14
# speedup vs baseline: 1.1808x; 1.0039x over previous
"""GPT3 parallel attention block on 8 Trainium2 NeuronCores.

Tensor-parallel over heads: each of the 8 cores owns 2 of the 16 heads.
Per core: QKV projection for its 768 channels, causal attention for its
2 heads x 2 batches, and the dense projection restricted to its head
columns, producing a partial [H, B*S] output. Partials are summed on the
host (the all-reduce of the reference sharding).

Layouts (device, per core):
  xT      [H, B*S]   fp16  hidden states transposed; token t = b*S + s
  wqkvT   [H, 768]   fp16  qkv weight slice, channels [q0 k0 v0 q1 k1 v1]
  qkvb    [768]      fp32  qkv bias slice (same channel order)
  dwT     [256, H]   fp16  dense weight slice, rows = (head, d) in-channels
  maskm   [128, 896] fp16  sliding causal mask master
  outT    [H, B*S]   fp16  partial output (out-channel major)

All matmuls run in fp16 operands / fp32 PSUM accumulation. Softmax is
unnormalized exp (no max subtraction; scores are O(1)) with the
denominator computed by an ones-matmul that replicates the row sum
across all 128 partitions, so the normalization is a plain elementwise
multiply by the DVE reciprocal.
"""

import math

import numpy as np

S, B, H, NH, D = 2048, 2, 2048, 16, 128
NCORES = 8
CHUNK = 512
N_CHUNKS = S // CHUNK  # 4
K_TILES = H // 128  # 16
SCALE = 1.0 / math.sqrt(float(D))  # coeff / (sqrt(d) * coeff)

_CACHE: dict = {}


def _build_program():
    import concourse.tile as tile
    from concourse import bacc, mybir
    from concourse.masks import make_identity

    fp16 = mybir.dt.float16
    fp32 = mybir.dt.float32

    nc = bacc.Bacc(
        "TRN2",
        target_bir_lowering=False,
        debug=False,
        enable_asserts=True,
        num_devices=NCORES,
    )
    xT = nc.dram_tensor("xT", [H, B * S], fp16, kind="ExternalInput").ap()
    wq = nc.dram_tensor("wqkvT", [H, 768], fp16, kind="ExternalInput").ap()
    qb = nc.dram_tensor("qkvb", [768], fp32, kind="ExternalInput").ap()
    dw = nc.dram_tensor("dwT", [256, H], fp16, kind="ExternalInput").ap()
    mask = nc.dram_tensor("maskm", [128, 896], fp16, kind="ExternalInput").ap()
    outT = nc.dram_tensor("outT", [H, B * S], fp16, kind="ExternalOutput").ap()

    with tile.TileContext(nc) as tc:
        with (
            tc.tile_pool(name="singles", bufs=1) as singles,
            tc.tile_pool(name="xk", bufs=36) as x_pool,
            tc.tile_pool(name="qt", bufs=4) as qt_pool,
            tc.tile_pool(name="kt", bufs=4) as kt_pool,
            tc.tile_pool(name="vv", bufs=4) as v_pool,
            tc.tile_pool(name="vt", bufs=3) as vt_pool,
            tc.tile_pool(name="pt", bufs=20) as pt_pool,
            tc.tile_pool(name="rec", bufs=2) as rec_pool,
            tc.tile_pool(name="cx", bufs=4) as cx_pool,
            tc.tile_pool(name="ost", bufs=6) as ost_pool,
            tc.tile_pool(name="ps_qkv", bufs=2, space="PSUM") as ps_qkv,
            tc.tile_pool(name="ps_misc", bufs=2, space="PSUM") as ps_misc,
            tc.tile_pool(name="ps_sc", bufs=2, space="PSUM") as ps_sc,
            tc.tile_pool(name="ps_ctx", bufs=2, space="PSUM") as ps_ctx,
        ):
            # --- one-time loads / constants (weight k-tiles split so the
            # first QKV accumulation can start before the full load lands)
            w_all = singles.tile([128, K_TILES, 768], fp16, tag="w_all")
            dw_all = singles.tile([128, 2, H], fp16, tag="dw_all")
            mask_t = singles.tile([128, 896], fp16, tag="mask_t")
            qb_t = singles.tile([128, 6], fp32, tag="qb_t")
            ident = singles.tile([128, 128], fp16, tag="ident")
            ones_t = singles.tile([128, 128], fp16, tag="ones_t")

            wq_v = wq.rearrange("(k p) c -> p k c", p=128)

            Ident = mybir.ActivationFunctionType.Identity
            Exp = mybir.ActivationFunctionType.Exp

            kT = {}
            Vb = {}
            qt = {}
            state = {}
            pt_gen = [0]  # first pass through the pt pool must write full tiles

            def load_x(b, j):
                tok0 = b * S + j * CHUNK
                xk = []
                for k in range(K_TILES):
                    xt = x_pool.tile([128, CHUNK], fp16, tag="xk", name="xk")
                    nc.sync.dma_start(
                        out=xt,
                        in_=xT[k * 128 : (k + 1) * 128, tok0 : tok0 + CHUNK],
                    )
                    xk.append(xt)
                return xk

            def stage1(b, j, xk):
                # q, k, v for chunk j of batch b, both heads, then V transpose
                if j == 0:
                    kT[b] = [
                        kt_pool.tile([128, S], fp16, tag="kt", name="kt")
                        for _ in range(2)
                    ]
                    Vb[b] = [
                        v_pool.tile([128, S], fp16, tag="vv", name="vv")
                        for _ in range(2)
                    ]
                vt = []
                qt[(b, j)] = []
                for h in range(2):
                    qtile = qt_pool.tile([128, CHUNK], fp16, tag="qt", name="qt")
                    vtile = vt_pool.tile([128, CHUNK], fp16, tag="vt", name="vt")
                    qt[(b, j)].append(qtile)
                    vt.append(vtile)
                    for which in range(3):  # q, k, v
                        ci = 3 * h + which
                        ps = ps_qkv.tile([128, CHUNK], fp32, tag="ps_qkv", name="ps")
                        for k in range(K_TILES):
                            nc.tensor.matmul(
                                out=ps,
                                lhsT=w_all[:, k, ci * 128 : (ci + 1) * 128],
                                rhs=xk[k],
                                start=(k == 0),
                                stop=(k == K_TILES - 1),
                            )
                        if which == 0:
                            dest = qtile
                        elif which == 1:
                            dest = kT[b][h][:, j * CHUNK : (j + 1) * CHUNK]
                        else:
                            dest = vtile
                        nc.vector.tensor_scalar_add(
                            out=dest, in0=ps, scalar1=qb_t[:, ci : ci + 1]
                        )
                for h in range(2):
                    tp = ps_misc.tile([128, CHUNK], fp16, tag="ps_misc", name="tp")
                    for ti in range(4):
                        nc.tensor.transpose(
                            out=tp[:, ti * 128 : (ti + 1) * 128],
                            in_=vt[h][:, ti * 128 : (ti + 1) * 128],
                            identity=ident,
                        )
                    nc.vector.tensor_copy(
                        out=Vb[b][h][:, j * CHUNK : (j + 1) * CHUNK], in_=tp
                    )

            def attn_a(b, j):
                accs = []
                for h in range(2):
                    n_t = 4 * j + 4
                    ctx = ps_ctx.tile([128, CHUNK], fp32, tag="ps_ctx", name="ctx")
                    pts = []
                    psums = []
                    quads = []
                    for i in range(n_t):
                        # diagonal tiles only need columns >= r; the masked
                        # rest of pt is zeroed by the mask multiply. The first
                        # generation of each pool slot must be written fully
                        # (stale SBUF can hold inf/NaN bit patterns).
                        rm = (i - 4 * j) * 128 if i >= 4 * j else 0
                        r = 0 if pt_gen[0] < 20 else rm
                        pt_gen[0] += 1
                        sc = ps_sc.tile([128, CHUNK], fp32, tag="ps_sc", name="sc")
                        nc.tensor.matmul(
                            out=sc[:, r:CHUNK],
                            lhsT=kT[b][h][:, i * 128 : (i + 1) * 128],
                            rhs=qt[(b, j)][h][:, r:CHUNK],
                            start=True,
                            stop=True,
                        )
                        pt = pt_pool.tile([128, CHUNK], fp16, tag="pt", name="pt")
                        nc.scalar.activation(
                            out=pt[:, r:CHUNK], in_=sc[:, r:CHUNK], func=Exp, scale=SCALE
                        )
                        if i >= 4 * j:
                            nc.vector.tensor_mul(
                                out=pt,
                                in0=pt,
                                in1=mask_t[:, 384 - rm : 384 - rm + CHUNK],
                            )
                        pts.append(pt)
                        if i % 2 == 1:
                            # pair- then quad-add on DVE: 4x fewer den matmuls
                            psum_t = pt_pool.tile(
                                [128, CHUNK], fp16, tag="pts", name="pts", bufs=4
                            )
                            nc.vector.tensor_add(
                                out=psum_t, in0=pts[i - 1], in1=pts[i]
                            )
                            psums.append(psum_t)
                            if i % 4 == 3:
                                q_t = pt_pool.tile(
                                    [128, CHUNK], fp16, tag="ptq", name="ptq", bufs=10
                                )
                                nc.vector.tensor_add(
                                    out=q_t, in0=psums[-2], in1=psums[-1]
                                )
                                quads.append(q_t)
                    for i in range(n_t):
                        nc.tensor.matmul(
                            out=ctx,
                            lhsT=Vb[b][h][:, i * 128 : (i + 1) * 128],
                            rhs=pts[i],
                            start=(i == 0),
                            stop=(i == n_t - 1),
                        )
                    accs.append((ctx, quads))
                state[(b, j, "acc")] = accs

            def attn_b(b, j):
                accs = state.pop((b, j, "acc"))
                ctx_chunk = []
                n_t = 4 * j + 4
                for h in range(2):
                    ctx, quads = accs[h]
                    den = ps_qkv.tile([128, CHUNK], fp32, tag="ps_qkv", name="den")
                    for p4 in range(n_t // 4):
                        nc.tensor.matmul(
                            out=den,
                            lhsT=ones_t,
                            rhs=quads[p4],
                            start=(p4 == 0),
                            stop=(p4 == n_t // 4 - 1),
                        )
                    rec = rec_pool.tile([128, CHUNK], fp32, tag="rec", name="rec")
                    nc.vector.reciprocal_approx_fast(out=rec, in_=den)
                    cxt = cx_pool.tile([128, CHUNK], fp16, tag="cx", name="cx")
                    nc.vector.tensor_mul(out=cxt, in0=ctx, in1=rec)
                    ctx_chunk.append(cxt)
                state[(b, j)] = ctx_chunk

            def dense(b, j):
                tok0 = b * S + j * CHUNK
                ctx_chunk = state.pop((b, j))
                for mi in range(16):
                    po = ps_misc.tile([128, CHUNK], fp32, tag="ps_misc", name="po")
                    for h in range(2):
                        nc.tensor.matmul(
                            out=po,
                            lhsT=dw_all[:, h, mi * 128 : (mi + 1) * 128],
                            rhs=ctx_chunk[h],
                            start=(h == 0),
                            stop=(h == 1),
                        )
                    ot = ost_pool.tile([128, CHUNK], fp16, tag="ost", name="ot")
                    nc.vector.tensor_copy(out=ot, in_=po)
                    nc.sync.dma_start(
                        out=outT[mi * 128 : (mi + 1) * 128, tok0 : tok0 + CHUNK],
                        in_=ot,
                    )

            chunks = [(b, j) for b in range(B) for j in range(N_CHUNKS)]

            # interleave the first x chunk with the weight k-tiles so the
            # first accumulation isn't gated on the full weight DMA
            nc.sync.dma_start(out=qb_t, in_=qb.rearrange("(g p) -> p g", p=128))
            nc.sync.dma_start(out=mask_t, in_=mask)
            make_identity(nc, ident)
            nc.vector.memset(ones_t, 1.0)
            # first channel column of the qkv weights, then the first x chunk,
            # then the remaining weight columns: the first accumulation group
            # can start as soon as the x tiles stream in
            nc.sync.dma_start(out=w_all[:, :, 0:128], in_=wq_v[:, :, 0:128])
            xk0 = []
            for k in range(K_TILES):
                xt = x_pool.tile([128, CHUNK], fp16, tag="xk", name="xk")
                nc.sync.dma_start(out=xt, in_=xT[k * 128 : (k + 1) * 128, 0:CHUNK])
                xk0.append(xt)
            for ci in range(1, 6):
                nc.sync.dma_start(
                    out=w_all[:, :, ci * 128 : (ci + 1) * 128],
                    in_=wq_v[:, :, ci * 128 : (ci + 1) * 128],
                )
            nc.sync.dma_start(out=dw_all, in_=dw.rearrange("(g p) o -> p g o", p=128))

            # software pipeline: stage1 of the next chunk is emitted between
            # attn and dense of the current chunk so the PE always has
            # independent matmul work while the softmax chain drains
            stage1(*chunks[0], xk0)
            xk_next = load_x(*chunks[1])
            for ci, (b, j) in enumerate(chunks):
                attn_a(b, j)
                if ci + 1 < len(chunks):
                    stage1(*chunks[ci + 1], xk_next)
                attn_b(b, j)
                if ci + 2 < len(chunks):
                    xk_next = load_x(*chunks[ci + 2])
                dense(b, j)
    nc.compile()
    return nc


def _get_program():
    if "nc" not in _CACHE:
        _CACHE["nc"] = _build_program()
    return _CACHE["nc"]


def _host_inputs(hidden_states, qkv_w, qkv_b, dense_w):
    xT = (
        np.ascontiguousarray(
            hidden_states.astype(np.float16).transpose(2, 1, 0)
        ).reshape(H, B * S)
    )
    maskm = (
        np.arange(128)[:, None] <= (np.arange(896)[None, :] - 384)
    ).astype(np.float16)
    in_maps = []
    for c in range(NCORES):
        wqkvT = np.ascontiguousarray(
            qkv_w[c * 768 : (c + 1) * 768].astype(np.float16).T
        )
        qkvb = np.ascontiguousarray(qkv_b[c * 768 : (c + 1) * 768]).astype(np.float32)
        dwT = np.ascontiguousarray(
            dense_w[:, c * 256 : (c + 1) * 256].astype(np.float16).T
        )
        in_maps.append(
            {
                "xT": xT,
                "wqkvT": wqkvT,
                "qkvb": qkvb,
                "dwT": dwT,
                "maskm": maskm,
            }
        )
    return in_maps


def run_spmd(in_maps, **kwargs):
    from concourse import bass_utils

    nc = _get_program()
    return bass_utils.run_bass_kernel_spmd(
        nc, in_maps, core_ids=list(range(NCORES)), **kwargs
    )


def kernel(hidden_states, attention_mask, qkv_w, qkv_b, dense_w, dense_b):
    hidden_states = np.asarray(hidden_states, dtype=np.float32)
    qkv_w = np.asarray(qkv_w, dtype=np.float32)
    qkv_b = np.asarray(qkv_b, dtype=np.float32)
    dense_w = np.asarray(dense_w, dtype=np.float32)
    dense_b = np.asarray(dense_b, dtype=np.float32)

    in_maps = _host_inputs(hidden_states, qkv_w, qkv_b, dense_w)
    res = run_spmd(in_maps)
    acc = np.zeros((H, B * S), np.float32)
    for r in res.results:
        acc += r["outT"].astype(np.float32)
    out = acc.reshape(H, B, S).transpose(2, 1, 0)
    return np.ascontiguousarray(out), dense_b


# revision 15
# speedup vs baseline: 1.1838x; 1.0025x over previous
"""GPT3 parallel attention block on 8 Trainium2 NeuronCores.

Tensor-parallel over heads: each of the 8 cores owns 2 of the 16 heads.
Per core: QKV projection for its 768 channels, causal attention for its
2 heads x 2 batches, and the dense projection restricted to its head
columns, producing a partial [H, B*S] output. Partials are summed on the
host (the all-reduce of the reference sharding).

Layouts (device, per core):
  xT      [H, B*S]   fp16  hidden states transposed; token t = b*S + s
  wqkvT   [H, 768]   fp16  qkv weight slice, channels [q0 k0 v0 q1 k1 v1]
  qkvb    [768]      fp32  qkv bias slice (same channel order)
  dwT     [256, H]   fp16  dense weight slice, rows = (head, d) in-channels
  maskm   [128, 896] fp16  sliding causal mask master
  outT    [H, B*S]   fp16  partial output (out-channel major)

All matmuls run in fp16 operands / fp32 PSUM accumulation. Softmax is
unnormalized exp (no max subtraction; scores are O(1)) with the
denominator computed by an ones-matmul that replicates the row sum
across all 128 partitions, so the normalization is a plain elementwise
multiply by the DVE reciprocal.
"""

import math

import numpy as np

S, B, H, NH, D = 2048, 2, 2048, 16, 128
NCORES = 8
CHUNK = 512
N_CHUNKS = S // CHUNK  # 4
K_TILES = H // 128  # 16
SCALE = 1.0 / math.sqrt(float(D))  # coeff / (sqrt(d) * coeff)

_CACHE: dict = {}


def _build_program():
    import concourse.tile as tile
    from concourse import bacc, mybir
    from concourse.masks import make_identity

    fp16 = mybir.dt.float16
    fp32 = mybir.dt.float32

    nc = bacc.Bacc(
        "TRN2",
        target_bir_lowering=False,
        debug=False,
        enable_asserts=True,
        num_devices=NCORES,
    )
    xT = nc.dram_tensor("xT", [H, B * S], fp16, kind="ExternalInput").ap()
    wq = nc.dram_tensor("wqkvT", [H, 768], fp16, kind="ExternalInput").ap()
    qb = nc.dram_tensor("qkvb", [768], fp32, kind="ExternalInput").ap()
    dw = nc.dram_tensor("dwT", [256, H], fp16, kind="ExternalInput").ap()
    mask = nc.dram_tensor("maskm", [128, 896], fp16, kind="ExternalInput").ap()
    outT = nc.dram_tensor("outT", [H, B * S], fp16, kind="ExternalOutput").ap()

    with tile.TileContext(nc) as tc:
        with (
            tc.tile_pool(name="singles", bufs=1) as singles,
            tc.tile_pool(name="xk", bufs=36) as x_pool,
            tc.tile_pool(name="qt", bufs=4) as qt_pool,
            tc.tile_pool(name="kt", bufs=4) as kt_pool,
            tc.tile_pool(name="vv", bufs=4) as v_pool,
            tc.tile_pool(name="vt", bufs=3) as vt_pool,
            tc.tile_pool(name="pt", bufs=20) as pt_pool,
            tc.tile_pool(name="rec", bufs=2) as rec_pool,
            tc.tile_pool(name="cx", bufs=4) as cx_pool,
            tc.tile_pool(name="ost", bufs=6) as ost_pool,
            tc.tile_pool(name="ps_qkv", bufs=2, space="PSUM") as ps_qkv,
            tc.tile_pool(name="ps_misc", bufs=2, space="PSUM") as ps_misc,
            tc.tile_pool(name="ps_sc", bufs=2, space="PSUM") as ps_sc,
            tc.tile_pool(name="ps_ctx", bufs=2, space="PSUM") as ps_ctx,
        ):
            # --- one-time loads / constants (weight k-tiles split so the
            # first QKV accumulation can start before the full load lands)
            w_all = singles.tile([128, K_TILES, 768], fp16, tag="w_all")
            dw_all = singles.tile([128, 2, H], fp16, tag="dw_all")
            mask_t = singles.tile([128, 896], fp16, tag="mask_t")
            qb_t = singles.tile([128, 6], fp32, tag="qb_t")
            ident = singles.tile([128, 128], fp16, tag="ident")
            ones_t = singles.tile([128, 128], fp16, tag="ones_t")

            wq_v = wq.rearrange("(k p) c -> p k c", p=128)

            Ident = mybir.ActivationFunctionType.Identity
            Exp = mybir.ActivationFunctionType.Exp

            kT = {}
            Vb = {}
            qt = {}
            state = {}
            pt_gen = [0]  # first pass through the pt pool must write full tiles

            def load_x(b, j):
                tok0 = b * S + j * CHUNK
                xk = []
                for k in range(K_TILES):
                    xt = x_pool.tile([128, CHUNK], fp16, tag="xk", name="xk")
                    nc.sync.dma_start(
                        out=xt,
                        in_=xT[k * 128 : (k + 1) * 128, tok0 : tok0 + CHUNK],
                    )
                    xk.append(xt)
                return xk

            def stage1(b, j, xk):
                # q, k, v for chunk j of batch b, both heads, then V transpose
                if j == 0:
                    kT[b] = [
                        kt_pool.tile([128, S], fp16, tag="kt", name="kt")
                        for _ in range(2)
                    ]
                    Vb[b] = [
                        v_pool.tile([128, S], fp16, tag="vv", name="vv")
                        for _ in range(2)
                    ]
                vt = []
                qt[(b, j)] = []
                for h in range(2):
                    qtile = qt_pool.tile([128, CHUNK], fp16, tag="qt", name="qt")
                    vtile = vt_pool.tile([128, CHUNK], fp16, tag="vt", name="vt")
                    qt[(b, j)].append(qtile)
                    vt.append(vtile)
                    for which in range(3):  # q, k, v
                        ci = 3 * h + which
                        ps = ps_qkv.tile([128, CHUNK], fp32, tag="ps_qkv", name="ps")
                        for k in range(K_TILES):
                            nc.tensor.matmul(
                                out=ps,
                                lhsT=w_all[:, k, ci * 128 : (ci + 1) * 128],
                                rhs=xk[k],
                                start=(k == 0),
                                stop=(k == K_TILES - 1),
                            )
                        if which == 0:
                            dest = qtile
                        elif which == 1:
                            dest = kT[b][h][:, j * CHUNK : (j + 1) * CHUNK]
                        else:
                            dest = vtile
                        nc.vector.tensor_scalar_add(
                            out=dest, in0=ps, scalar1=qb_t[:, ci : ci + 1]
                        )
                for h in range(2):
                    tp = ps_misc.tile([128, CHUNK], fp16, tag="ps_misc", name="tp")
                    for ti in range(4):
                        nc.tensor.transpose(
                            out=tp[:, ti * 128 : (ti + 1) * 128],
                            in_=vt[h][:, ti * 128 : (ti + 1) * 128],
                            identity=ident,
                        )
                    nc.vector.tensor_copy(
                        out=Vb[b][h][:, j * CHUNK : (j + 1) * CHUNK], in_=tp
                    )

            def attn_a(b, j):
                accs = []
                for h in range(2):
                    n_t = 4 * j + 4
                    ctx = ps_ctx.tile([128, CHUNK], fp32, tag="ps_ctx", name="ctx")
                    pts = []
                    psums = []
                    quads = []
                    for i in range(n_t):
                        # diagonal tiles only need columns >= r; the masked
                        # rest of pt is zeroed by the mask multiply. The first
                        # generation of each pool slot must be written fully
                        # (stale SBUF can hold inf/NaN bit patterns).
                        rm = (i - 4 * j) * 128 if i >= 4 * j else 0
                        r = 0 if pt_gen[0] < 20 else rm
                        pt_gen[0] += 1
                        sc = ps_sc.tile([128, CHUNK], fp32, tag="ps_sc", name="sc")
                        nc.tensor.matmul(
                            out=sc[:, r:CHUNK],
                            lhsT=kT[b][h][:, i * 128 : (i + 1) * 128],
                            rhs=qt[(b, j)][h][:, r:CHUNK],
                            start=True,
                            stop=True,
                        )
                        pt = pt_pool.tile([128, CHUNK], fp16, tag="pt", name="pt")
                        nc.scalar.activation(
                            out=pt[:, r:CHUNK], in_=sc[:, r:CHUNK], func=Exp, scale=SCALE
                        )
                        if i >= 4 * j:
                            nc.vector.tensor_mul(
                                out=pt,
                                in0=pt,
                                in1=mask_t[:, 384 - rm : 384 - rm + CHUNK],
                            )
                        pts.append(pt)
                        if i % 2 == 1:
                            # pair- then quad-add on DVE: 4x fewer den matmuls
                            psum_t = pt_pool.tile(
                                [128, CHUNK], fp16, tag="pts", name="pts", bufs=4
                            )
                            nc.vector.tensor_add(
                                out=psum_t, in0=pts[i - 1], in1=pts[i]
                            )
                            psums.append(psum_t)
                            if i % 4 == 3:
                                q_t = pt_pool.tile(
                                    [128, CHUNK], fp16, tag="ptq", name="ptq", bufs=10
                                )
                                nc.vector.tensor_add(
                                    out=q_t, in0=psums[-2], in1=psums[-1]
                                )
                                quads.append(q_t)
                    for i in range(n_t):
                        rv = (i - 4 * j) * 128 if i > 4 * j else 0
                        nc.tensor.matmul(
                            out=ctx[:, rv:CHUNK],
                            lhsT=Vb[b][h][:, i * 128 : (i + 1) * 128],
                            rhs=pts[i][:, rv:CHUNK],
                            start=(i == 0),
                            stop=(i == n_t - 1),
                        )
                    accs.append((ctx, quads))
                state[(b, j, "acc")] = accs

            def attn_b(b, j):
                accs = state.pop((b, j, "acc"))
                ctx_chunk = []
                n_t = 4 * j + 4
                for h in range(2):
                    ctx, quads = accs[h]
                    den = ps_qkv.tile([128, CHUNK], fp32, tag="ps_qkv", name="den")
                    for p4 in range(n_t // 4):
                        nc.tensor.matmul(
                            out=den,
                            lhsT=ones_t,
                            rhs=quads[p4],
                            start=(p4 == 0),
                            stop=(p4 == n_t // 4 - 1),
                        )
                    rec = rec_pool.tile([128, CHUNK], fp32, tag="rec", name="rec")
                    nc.vector.reciprocal_approx_fast(out=rec, in_=den)
                    cxt = cx_pool.tile([128, CHUNK], fp16, tag="cx", name="cx")
                    nc.vector.tensor_mul(out=cxt, in0=ctx, in1=rec)
                    ctx_chunk.append(cxt)
                state[(b, j)] = ctx_chunk

            def dense(b, j):
                tok0 = b * S + j * CHUNK
                ctx_chunk = state.pop((b, j))
                for mi in range(16):
                    po = ps_misc.tile([128, CHUNK], fp32, tag="ps_misc", name="po")
                    for h in range(2):
                        nc.tensor.matmul(
                            out=po,
                            lhsT=dw_all[:, h, mi * 128 : (mi + 1) * 128],
                            rhs=ctx_chunk[h],
                            start=(h == 0),
                            stop=(h == 1),
                        )
                    ot = ost_pool.tile([128, CHUNK], fp16, tag="ost", name="ot")
                    nc.vector.tensor_copy(out=ot, in_=po)
                    nc.sync.dma_start(
                        out=outT[mi * 128 : (mi + 1) * 128, tok0 : tok0 + CHUNK],
                        in_=ot,
                    )

            chunks = [(b, j) for b in range(B) for j in range(N_CHUNKS)]

            # interleave the first x chunk with the weight k-tiles so the
            # first accumulation isn't gated on the full weight DMA
            nc.sync.dma_start(out=qb_t, in_=qb.rearrange("(g p) -> p g", p=128))
            nc.sync.dma_start(out=mask_t, in_=mask)
            make_identity(nc, ident)
            nc.vector.memset(ones_t, 1.0)
            # head-0 weights and the first x chunk land first so head-0's
            # QKV groups start while head-1 weights stream in behind them
            xk0 = []
            for k in range(K_TILES):
                nc.sync.dma_start(out=w_all[:, k, 0:384], in_=wq_v[:, k, 0:384])
                xt = x_pool.tile([128, CHUNK], fp16, tag="xk", name="xk")
                nc.sync.dma_start(out=xt, in_=xT[k * 128 : (k + 1) * 128, 0:CHUNK])
                xk0.append(xt)
            for k in range(K_TILES):
                nc.sync.dma_start(out=w_all[:, k, 384:768], in_=wq_v[:, k, 384:768])
            nc.sync.dma_start(out=dw_all, in_=dw.rearrange("(g p) o -> p g o", p=128))

            # software pipeline: stage1 of the next chunk is emitted between
            # attn and dense of the current chunk so the PE always has
            # independent matmul work while the softmax chain drains
            stage1(*chunks[0], xk0)
            xk_next = load_x(*chunks[1])
            for ci, (b, j) in enumerate(chunks):
                attn_a(b, j)
                if ci + 1 < len(chunks):
                    stage1(*chunks[ci + 1], xk_next)
                attn_b(b, j)
                if ci + 2 < len(chunks):
                    xk_next = load_x(*chunks[ci + 2])
                dense(b, j)
    nc.compile()
    return nc


def _get_program():
    if "nc" not in _CACHE:
        _CACHE["nc"] = _build_program()
    return _CACHE["nc"]


def _host_inputs(hidden_states, qkv_w, qkv_b, dense_w):
    xT = (
        np.ascontiguousarray(
            hidden_states.astype(np.float16).transpose(2, 1, 0)
        ).reshape(H, B * S)
    )
    maskm = (
        np.arange(128)[:, None] <= (np.arange(896)[None, :] - 384)
    ).astype(np.float16)
    in_maps = []
    for c in range(NCORES):
        wqkvT = np.ascontiguousarray(
            qkv_w[c * 768 : (c + 1) * 768].astype(np.float16).T
        )
        qkvb = np.ascontiguousarray(qkv_b[c * 768 : (c + 1) * 768]).astype(np.float32)
        dwT = np.ascontiguousarray(
            dense_w[:, c * 256 : (c + 1) * 256].astype(np.float16).T
        )
        in_maps.append(
            {
                "xT": xT,
                "wqkvT": wqkvT,
                "qkvb": qkvb,
                "dwT": dwT,
                "maskm": maskm,
            }
        )
    return in_maps


def run_spmd(in_maps, **kwargs):
    from concourse import bass_utils

    nc = _get_program()
    return bass_utils.run_bass_kernel_spmd(
        nc, in_maps, core_ids=list(range(NCORES)), **kwargs
    )


def kernel(hidden_states, attention_mask, qkv_w, qkv_b, dense_w, dense_b):
    hidden_states = np.asarray(hidden_states, dtype=np.float32)
    qkv_w = np.asarray(qkv_w, dtype=np.float32)
    qkv_b = np.asarray(qkv_b, dtype=np.float32)
    dense_w = np.asarray(dense_w, dtype=np.float32)
    dense_b = np.asarray(dense_b, dtype=np.float32)

    in_maps = _host_inputs(hidden_states, qkv_w, qkv_b, dense_w)
    res = run_spmd(in_maps)
    acc = np.zeros((H, B * S), np.float32)
    for r in res.results:
        acc += r["outT"].astype(np.float32)
    out = acc.reshape(H, B, S).transpose(2, 1, 0)
    return np.ascontiguousarray(out), dense_b
